# revision 18
# baseline (speedup 1.0000x reference)
"""Trainium2 Bass kernel for the correlation-map embedding module (v17).

Math (per (b, nf) pair):
  f1d = bilinear_down28(feature_i[b, nf])                  # [C, 28, 28]
  f2sel[c, k] = bilinear sample of feature_j[b, nf] at the K knn grid points
  corr[k, :, :] = relu(sum_c f2sel[c, k] * f1d[c, :, :])   # [K, 28, 28]
  out[k] = corr[k] / sum_hw(exp(corr[k])) * 10

Structure (lineage: v14 Tile 26.4us, v16 raw bass 25.1us):
  - host prep does both bilinear resamples; device loads f1d [C, 784] +
    f2sel [C, K] per pair (1.4MB f16) and stores 10*relu(corr) f16 plus
    the f32 exp-sum denominators bit-packed into the same 393-f32-per-
    pair output rows (1.2MB); the normalize (o/s) happens on the host
    during unshard.
  - RAW bass (one Block, 9 semaphores): v14's TileContext spent ~6us
    zeroing tick-semaphores at exit and ~8us in EVENT_SEMAPHORE waits.
  - HW constraint found by bisection (hangs the device, CoreSim-silent):
    ACT and DVE must NOT read the same PSUM bank concurrently.  The DVE
    trails the ACT by one pair (vector waits ae >= p+1); with 4 PSUM
    pair-buffers (all 8 banks) both engines still run fully parallel on
    different pairs and the PE runs up to 3 pairs ahead.
  - per pair: 2 fp16 128x128x392 matmuls (f32 PSUM); ACT computes
    s[p] = sum_q exp(corr) via accum_out (relu dropped from the exp
    argument: the sum is dominated by exp(corr_max) ~ e^15..e^47, rel
    err <= 1.3e-3); DVE computes o = max(10*corr, 0) -> f16 from PSUM.
  - per-pair loads and stores (6+6 DMAs on the sync queue): the first
    233KB load bounds the ~3us DMA-completion lead-in, later pairs land
    ~0.7us apart well ahead of the ACT-paced pipeline, and the last
    201KB store minimizes the completion-latency tail.
  - dummy exp at t=0 prefetches the ACT table during the DMA lead-in.

Sharding: pure data parallel - batch dim (16) split across 8 cores, 2 each.
"""

import numpy as np

# hardcoded problem shapes (grading calls kernel(**inputs) standalone)
B, NF, C, H, W = 16, 3, 128, 56, 56
G = 28
K = 128
NCORES = 8
BPC = B // NCORES  # 2
NPAIR = NF * BPC  # 6
P = 128
QH = G * G // 2  # 392 psum columns per bank
PAIRW = K + G * G  # 912 f16 per partition per pair
OW = QH + 1  # 393 f32 out columns per pair: 784 f16 o-values | 1 f32 s

_CACHE = {}


def _axis_coords(n_in):
    # float32 arithmetic to match the jax reference bit-for-bit
    src = np.arange(G, dtype=np.float32) * np.float32((n_in - 1) / (G - 1))
    i0 = np.clip(np.floor(src).astype(np.int32), 0, n_in - 2)
    w = (src - i0.astype(np.float32)).astype(np.float32)
    return i0, w


def _downsample28(x):
    """align_corners bilinear [..., H, W] f32 -> [..., 28, 28] f32."""
    i0h, wh = _axis_coords(H)
    i0w, ww = _axis_coords(W)
    r = x[..., i0h, :] * (1.0 - wh)[:, None] + x[..., i0h + 1, :] * wh[:, None]
    return r[..., i0w] * (1.0 - ww) + r[..., i0w + 1] * ww


def _build_bass():
    import concourse.bacc as bacc
    from concourse import mybir

    f32 = mybir.dt.float32
    bf16 = mybir.dt.bfloat16
    f16 = mybir.dt.float16
    AF = mybir.ActivationFunctionType
    OP = mybir.AluOpType

    nc = bacc.Bacc()
    combo_d = nc.dram_tensor("combo", [P, NPAIR, PAIRW], f16, kind="ExternalInput")
    out_d = nc.dram_tensor("out", [P, NPAIR, OW], f32, kind="ExternalOutput")

    with (
        nc.sbuf_tensor([P, NPAIR, PAIRW], f16) as combo,
        nc.sbuf_tensor([P, NPAIR, OW], f32) as o,
        nc.sbuf_tensor([P, 4, 2, QH], bf16) as e,
        nc.sbuf_tensor([P, 1], f32) as scratch,
        nc.psum_tensor([P, 4, 2, 512], f32) as ps,
        nc.semaphore() as ld0,
        nc.semaphore() as ld1,
        nc.semaphore() as ld2,
        nc.semaphore() as ld3,
        nc.semaphore() as ld4,
        nc.semaphore() as ld5,
        nc.semaphore() as ms,
        nc.semaphore() as mm,
        nc.semaphore() as ae,
        nc.semaphore() as ve,
        nc.semaphore() as st,
        nc.Block(no_gpsimd_drain=True) as block,
    ):
        lds = [ld0, ld1, ld2, ld3, ld4, ld5]
        of16 = o.bitcast(f16)  # [P, NPAIR, 786]: 784 f16 o-values | 4B f32 s

        @block.sync
        def _(sync):
            for p in range(NPAIR):
                sync.dma_start(out=combo[:, p], in_=combo_d[:, p]).then_inc(
                    lds[p], 16
                )
            for p in range(NPAIR):
                # ve >= p+1 also implies ae >= p+1 (DVE trails ACT), so the
                # pair's s slot is written too
                sync.wait_ge(ve, p + 1)
                sync.dma_start(out=out_d[:, p], in_=o[:, p]).then_inc(st, 16)
            sync.wait_ge(st, 16 * NPAIR)

        @block.tensor
        def _(tensor):
            for p in range(NPAIR):
                tensor.wait_ge(lds[p], 16)
                if p >= 4:
                    # ve >= p-2 implies DVE (and transitively ACT) finished
                    # pair p-4, freeing psum buffer (p-4) % 4 == p % 4
                    tensor.wait_ge(ve, p - 3)
                for h in range(2):
                    ins = nc.tensor.matmul(
                        ps[:, p % 4, h, :QH],
                        lhsT=combo[:, p, :K],
                        rhs=combo[:, p, K + h * QH : K + (h + 1) * QH],
                        start=True,
                        stop=True,
                    )
                ins.then_inc(mm, 1)

        @block.scalar
        def _(scalar):
            # dummy exp at t=0 pulls the ACT_TABLE_LOAD off the critical path
            scalar.wait_ge(ms, 1)
            nc.scalar.activation(scratch[:, :], scratch[:, :], AF.Exp, bias=0.0)
            for p in range(NPAIR):
                scalar.wait_ge(mm, p + 1)
                # then_inc rides the last walrus-lowered instruction (the
                # accumulator read), so ae => PSUM free AND s written
                nc.scalar.activation(
                    e[:, p % 4],
                    ps[:, p % 4, :, :QH],
                    AF.Exp,
                    bias=0.0,
                    accum_out=o[:, p, QH : QH + 1],
                ).then_inc(ae, 1)

        @block.vector
        def _(vector):
            vector.memset(scratch[:, :], 0.0).then_inc(ms, 1)
            for p in range(NPAIR):
                # ae >= p+1: never read a PSUM bank while ACT is reading it
                # (concurrent ACT+DVE reads of one bank hang the device)
                vector.wait_ge(ae, p + 1)
                oap = of16[:, p, : 2 * QH].rearrange("p (h q) -> p h q", h=2)
                nc.vector.tensor_scalar(
                    oap, ps[:, p % 4, :, :QH], 10.0, 0.0,
                    op0=OP.mult, op1=OP.max,
                ).then_inc(ve, 1)

    return nc


def _get_bass():
    if "nc" not in _CACHE:
        nc = _build_bass()
        if not nc.is_finalized():
            nc.finalize()
        _CACHE["nc"] = nc
    return _CACHE["nc"]


def _prepare_in_maps(feature_i, feature_j, knn_inds):
    fi = np.asarray(feature_i, dtype=np.float32)  # [B, NF, C, H, W]
    fj = np.asarray(feature_j, dtype=np.float32)
    knn = np.asarray(knn_inds).astype(np.int64)  # [NF, K, 2]

    f1d = _downsample28(fi).reshape(B, NF, C, G * G)
    f2d = _downsample28(fj)  # [B, NF, C, 28, 28]
    # f2sel[b,nf,c,k] = f2d[b,nf,c,h2,w2] with h2 = knn[nf,k,1], w2 = knn[nf,k,0]
    f2sel = np.empty((B, NF, C, K), np.float32)
    for nf in range(NF):
        f2sel[:, nf] = f2d[:, nf][:, :, knn[nf, :, 1], knn[nf, :, 0]]

    # pair p = nf * BPC + b; device layout [C(part), pair, K | 784]
    combo = np.empty((NCORES, P, NF, BPC, PAIRW), np.float16)
    combo[..., :K] = f2sel.reshape(NCORES, BPC, NF, C, K).transpose(0, 3, 2, 1, 4)
    combo[..., K:] = f1d.reshape(NCORES, BPC, NF, C, G * G).transpose(0, 3, 2, 1, 4)
    combo = combo.reshape(NCORES, P, NPAIR, PAIRW)
    return [{"combo": np.ascontiguousarray(combo[c])} for c in range(NCORES)]


def kernel(feature_i, feature_j, mask, optical_flow, knn_inds):
    from concourse import bass_utils

    nc = _get_bass()
    in_maps = _prepare_in_maps(feature_i, feature_j, knn_inds)

    res = bass_utils.run_bass_kernel_spmd(nc, in_maps, core_ids=list(range(NCORES)))
    # [core, K(part), pair, 393 f32]: cols :392 = 784 f16 of 10*relu(corr),
    # col 392 = f32 sum_q exp(corr)
    buf = np.stack([res.results[c]["out"] for c in range(NCORES)], axis=0)
    o = buf[..., :QH].copy().view(np.float16).astype(np.float32)  # [core,K,pair,784]
    s = buf[..., QH]  # [core, K, pair]
    out = o / s[..., None]
    # [core, K(part), pair=(nf,b), 784] -> [B, NF, K, 28, 28]
    out = out.reshape(NCORES, K, NF, BPC, G * G).transpose(0, 3, 2, 1, 4)
    return np.ascontiguousarray(out.reshape(B, NF, K, G, G))


# revision 19
# speedup vs baseline: 1.0682x; 1.0682x over previous
"""Trainium2 Bass kernel for the correlation-map embedding module (v17).

Math (per (b, nf) pair):
  f1d = bilinear_down28(feature_i[b, nf])                  # [C, 28, 28]
  f2sel[c, k] = bilinear sample of feature_j[b, nf] at the K knn grid points
  corr[k, :, :] = relu(sum_c f2sel[c, k] * f1d[c, :, :])   # [K, 28, 28]
  out[k] = corr[k] / sum_hw(exp(corr[k])) * 10

Structure (lineage: v14 Tile 26.4us, v16 raw bass 25.1us):
  - host prep does both bilinear resamples; device loads f1d [C, 784] +
    f2sel [C, K] per pair (1.4MB f16) and stores 10*relu(corr) f16 plus
    the f32 exp-sum denominators bit-packed into the same 393-f32-per-
    pair output rows (1.2MB); the normalize (o/s) happens on the host
    during unshard.
  - RAW bass (one Block, 9 semaphores): v14's TileContext spent ~6us
    zeroing tick-semaphores at exit and ~8us in EVENT_SEMAPHORE waits.
  - HW constraint found by bisection (hangs the device, CoreSim-silent):
    ACT and DVE must NOT read the same PSUM bank concurrently.  The DVE
    trails the ACT by one pair (vector waits ae >= p+1); with 4 PSUM
    pair-buffers (all 8 banks) both engines still run fully parallel on
    different pairs and the PE runs up to 3 pairs ahead.
  - per pair: 2 fp16 128x128x392 matmuls (f32 PSUM); ACT computes
    s[p] = sum_q exp(corr) via accum_out (relu dropped from the exp
    argument: the sum is dominated by exp(corr_max) ~ e^15..e^47, rel
    err <= 1.3e-3); DVE computes o = max(10*corr, 0) -> f16 from PSUM.
  - per-pair loads and stores (6+6 DMAs on the sync queue): the first
    233KB load bounds the ~3us DMA-completion lead-in, later pairs land
    ~0.7us apart well ahead of the ACT-paced pipeline, and the last
    201KB store minimizes the completion-latency tail.
  - dummy exp at t=0 prefetches the ACT table during the DMA lead-in.

Sharding: pure data parallel - batch dim (16) split across 8 cores, 2 each.
"""

import numpy as np

# hardcoded problem shapes (grading calls kernel(**inputs) standalone)
B, NF, C, H, W = 16, 3, 128, 56, 56
G = 28
K = 128
NCORES = 8
BPC = B // NCORES  # 2
NPAIR = NF * BPC  # 6
P = 128
QH = G * G // 2  # 392 psum columns per bank
PAIRW = K + G * G  # 912 f16 per partition per pair
OW = QH + 1  # 393 f32 out columns per pair: 784 f16 o-values | 1 f32 s

_CACHE = {}


def _axis_coords(n_in):
    # float32 arithmetic to match the jax reference bit-for-bit
    src = np.arange(G, dtype=np.float32) * np.float32((n_in - 1) / (G - 1))
    i0 = np.clip(np.floor(src).astype(np.int32), 0, n_in - 2)
    w = (src - i0.astype(np.float32)).astype(np.float32)
    return i0, w


def _downsample28(x):
    """align_corners bilinear [..., H, W] f32 -> [..., 28, 28] f32."""
    i0h, wh = _axis_coords(H)
    i0w, ww = _axis_coords(W)
    r = x[..., i0h, :] * (1.0 - wh)[:, None] + x[..., i0h + 1, :] * wh[:, None]
    return r[..., i0w] * (1.0 - ww) + r[..., i0w + 1] * ww


def _build_bass():
    import concourse.bacc as bacc
    from concourse import mybir

    f32 = mybir.dt.float32
    bf16 = mybir.dt.bfloat16
    f16 = mybir.dt.float16
    AF = mybir.ActivationFunctionType
    OP = mybir.AluOpType

    nc = bacc.Bacc()
    combo_d = nc.dram_tensor("combo", [P, NPAIR, PAIRW], f16, kind="ExternalInput")
    out_d = nc.dram_tensor("out", [P, NPAIR, OW], f32, kind="ExternalOutput")

    LOADG = [(0, 1), (1, 3), (3, 6)]  # pair ranges per load DMA
    PAIR_LD = [1, 2, 2, 3, 3, 3]  # load index (1-based) pair p depends on

    with (
        nc.sbuf_tensor([P, NPAIR, PAIRW], f16) as combo,
        nc.sbuf_tensor([P, NPAIR, OW], f32) as o,
        nc.sbuf_tensor([P, 3, 2, QH], bf16) as e,
        nc.sbuf_tensor([P, 1], f32) as scratch,
        nc.psum_tensor([P, 3, 2, 512], f32) as ps,
        nc.semaphore() as ld0,
        nc.semaphore() as ld1,
        nc.semaphore() as ld2,
        nc.semaphore() as ms,
        nc.semaphore() as mm,
        nc.semaphore() as ae,
        nc.semaphore() as ve,
        nc.semaphore() as st,
        nc.Block() as block,
    ):
        lds = [ld0, ld1, ld2]
        of16 = o.bitcast(f16)  # [P, NPAIR, 786]: 784 f16 o-values | 4B f32 s

        @block.sync
        def _(sync):
            for i, (a, b) in enumerate(LOADG):
                sync.dma_start(out=combo[:, a:b], in_=combo_d[:, a:b]).then_inc(
                    lds[i], 16
                )
            # ve >= 3/6 also implies ae >= 3/6 (DVE trails ACT), so the
            # s slots of the group are written too
            sync.wait_ge(ve, 3)
            sync.dma_start(out=out_d[:, 0:3], in_=o[:, 0:3]).then_inc(st, 16)
            sync.wait_ge(ve, 6)
            sync.dma_start(out=out_d[:, 3:6], in_=o[:, 3:6]).then_inc(st, 16)
            sync.wait_ge(st, 32)

        @block.tensor
        def _(tensor):
            for p in range(NPAIR):
                if p == 0 or PAIR_LD[p] != PAIR_LD[p - 1]:
                    tensor.wait_ge(lds[PAIR_LD[p] - 1], 16)
                if p >= 3:
                    # ve >= p-2 implies DVE (and transitively ACT) finished
                    # pair p-3, freeing psum buffer (p-3) % 3 == p % 3
                    tensor.wait_ge(ve, p - 2)
                for h in range(2):
                    ins = nc.tensor.matmul(
                        ps[:, p % 3, h, :QH],
                        lhsT=combo[:, p, :K],
                        rhs=combo[:, p, K + h * QH : K + (h + 1) * QH],
                        start=True,
                        stop=True,
                    )
                ins.then_inc(mm, 1)

        @block.scalar
        def _(scalar):
            # dummy exp at t=0 pulls the ACT_TABLE_LOAD off the critical path
            scalar.wait_ge(ms, 1)
            nc.scalar.activation(scratch[:, :], scratch[:, :], AF.Exp, bias=0.0)
            for p in range(NPAIR):
                scalar.wait_ge(mm, p + 1)
                # then_inc rides the last walrus-lowered instruction (the
                # accumulator read), so ae => PSUM free AND s written
                nc.scalar.activation(
                    e[:, p % 3],
                    ps[:, p % 3, :, :QH],
                    AF.Exp,
                    bias=0.0,
                    accum_out=o[:, p, QH : QH + 1],
                ).then_inc(ae, 1)

        @block.vector
        def _(vector):
            vector.memset(scratch[:, :], 0.0).then_inc(ms, 1)
            for p in range(NPAIR):
                # ae >= p+1: never read a PSUM bank while ACT is reading it
                # (concurrent ACT+DVE reads of one bank hang the device)
                vector.wait_ge(ae, p + 1)
                oap = of16[:, p, : 2 * QH].rearrange("p (h q) -> p h q", h=2)
                nc.vector.tensor_scalar(
                    oap, ps[:, p % 3, :, :QH], 10.0, 0.0,
                    op0=OP.mult, op1=OP.max,
                ).then_inc(ve, 1)

    return nc


def _get_bass():
    if "nc" not in _CACHE:
        nc = _build_bass()
        if not nc.is_finalized():
            nc.finalize()
        _CACHE["nc"] = nc
    return _CACHE["nc"]


def _prepare_in_maps(feature_i, feature_j, knn_inds):
    fi = np.asarray(feature_i, dtype=np.float32)  # [B, NF, C, H, W]
    fj = np.asarray(feature_j, dtype=np.float32)
    knn = np.asarray(knn_inds).astype(np.int64)  # [NF, K, 2]

    f1d = _downsample28(fi).reshape(B, NF, C, G * G)
    f2d = _downsample28(fj)  # [B, NF, C, 28, 28]
    # f2sel[b,nf,c,k] = f2d[b,nf,c,h2,w2] with h2 = knn[nf,k,1], w2 = knn[nf,k,0]
    f2sel = np.empty((B, NF, C, K), np.float32)
    for nf in range(NF):
        f2sel[:, nf] = f2d[:, nf][:, :, knn[nf, :, 1], knn[nf, :, 0]]

    # pair p = nf * BPC + b; device layout [C(part), pair, K | 784]
    combo = np.empty((NCORES, P, NF, BPC, PAIRW), np.float16)
    combo[..., :K] = f2sel.reshape(NCORES, BPC, NF, C, K).transpose(0, 3, 2, 1, 4)
    combo[..., K:] = f1d.reshape(NCORES, BPC, NF, C, G * G).transpose(0, 3, 2, 1, 4)
    combo = combo.reshape(NCORES, P, NPAIR, PAIRW)
    return [{"combo": np.ascontiguousarray(combo[c])} for c in range(NCORES)]


def kernel(feature_i, feature_j, mask, optical_flow, knn_inds):
    from concourse import bass_utils

    nc = _get_bass()
    in_maps = _prepare_in_maps(feature_i, feature_j, knn_inds)

    res = bass_utils.run_bass_kernel_spmd(nc, in_maps, core_ids=list(range(NCORES)))
    # [core, K(part), pair, 393 f32]: cols :392 = 784 f16 of 10*relu(corr),
    # col 392 = f32 sum_q exp(corr)
    buf = np.stack([res.results[c]["out"] for c in range(NCORES)], axis=0)
    o = buf[..., :QH].copy().view(np.float16).astype(np.float32)  # [core,K,pair,784]
    s = buf[..., QH]  # [core, K, pair]
    out = o / s[..., None]
    # [core, K(part), pair=(nf,b), 784] -> [B, NF, K, 28, 28]
    out = out.reshape(NCORES, K, NF, BPC, G * G).transpose(0, 3, 2, 1, 4)
    return np.ascontiguousarray(out.reshape(B, NF, K, G, G))
